# revision 19
# baseline (speedup 1.0000x reference)
"""Trainium2 Bass kernel for nn_DifferentiableTortuosity.

Math: 50 iterations of D = min(D, (conv4(D)/4 + 1) * ip) on a (B,512,512)
grid, sampled at start_coords. Information propagates 1 cell/iteration, so
D^50[start] depends only on cells within L1 distance 50 of start: a 101x101
window centered at start is exact. Out-of-map cells (window sticking past the
map edge) behave exactly like the reference's zero padding as long as they
start at D=0: eff >= 0 everywhere, so min keeps them pinned at 0.

Layout per core: 8 batch windows stacked along the free dim in 104-col slots
of one [101, 832] fp32 SBUF tile (3 zero guard cols between slots). Start is
always at local (50, 50+104*b), so one SPMD program serves all cores.

Per iteration:
  PE : V = tridiag @ D            (vertical neighbor sum; fp32, bit-exact)
  DVE: H[c] = D[c-1] + D[c+1]     (horizontal, shifted-AP add)
  DVE: N = V + H
  DVE: eff = (N + 4) * (ip/4)     (scalar_tensor_tensor; == (N/4+1)*ip in fp32)
  DVE: D = min(D, eff)
"""
import numpy as np

B_FULL = 64
H = 512
W = 512
NCORES = 8
BPC = B_FULL // NCORES  # 8 batches per core
R = 50
WIN = 2 * R + 1   # 101
SLOT = 104        # window cols + 3 guard cols
WCOLS = SLOT * BPC  # 832
NUM_ITER = 50
EPS = 1e-06

_COMPILED = {}

# active-window floor (instruction overhead dominates below this width)
W_FLOOR = 25
# run eff (and min) on GpSimd while the active width is at least this
POOL_EFF_MIN_W = 41

# v3 config: number of batch groups and per-group engine picks
V3_GROUPS = 2
V3_EFF_ENG = ("pool", "pool")   # per group: "pool" | "dve"
V3_MIN_ENG = ("dve", "pool")
V3_H4_ENG = ("dve", "pool")
V3_FLOOR = 1
V3_PE_WARM = 0  # filler matmuls per iter to keep PE ramped
V3_MERGE_W = 0  # merge groups into one chain when w <= this (0 = never)


def _build_program_v3(n_iter=NUM_ITER):
    """Two independent batch-group chains interleaved across engines, with
    the active column band shrinking to the bare dependency cone (floor 1).
    Per group and iteration: PE tridiag matmul (V), DVE stt (H4=(l+4)+r),
    DVE add (N=V+H4), eff=N*ip4 and min on configurable engines."""
    import concourse.bacc as bacc
    import concourse.tile as tile
    from concourse import mybir

    nc = bacc.Bacc("TRN2", target_bir_lowering=False, debug=False,
                   num_devices=NCORES)
    pm_in = nc.declare_dram_parameter("pmwin", [WIN, WCOLS], mybir.dt.float32,
                                      isOutput=False)
    d0_in = nc.declare_dram_parameter("d0win", [WIN, WCOLS], mybir.dt.float32,
                                      isOutput=False)
    pl_out = nc.declare_dram_parameter("plens", [1, BPC], mybir.dt.float32,
                                       isOutput=True)
    warm_out = None
    if V3_PE_WARM:
        warm_out = nc.declare_dram_parameter(
            "warm_out", [1, 1], mybir.dt.float32, isOutput=True)

    tri_np = np.zeros((WIN, WIN), dtype=np.float32)
    for i in range(WIN):
        if i > 0:
            tri_np[i - 1, i] = 1.0
        if i < WIN - 1:
            tri_np[i + 1, i] = 1.0
    tri_dram = nc.inline_tensor(tri_np, "tri")

    G = V3_GROUPS
    GB = BPC // G  # batches per group

    GW = (BPC // G) * SLOT  # columns per group

    with tile.TileContext(nc) as tc:
        with (
            tc.tile_pool(name="state", bufs=1) as state,
            tc.tile_pool(name="tmp", bufs=3) as tmp,
            tc.tile_pool(name="ps", bufs=2, space="PSUM") as ps,
        ):
            D = state.tile([WIN, WCOLS], mybir.dt.float32)
            IP4 = state.tile([WIN, WCOLS], mybir.dt.float32)
            TRI = state.tile([WIN, WIN], mybir.dt.float32)
            nc.sync.dma_start(out=TRI[:], in_=tri_dram[:])
            nc.sync.dma_start(out=D[:], in_=d0_in[:])
            nc.sync.dma_start(out=IP4[:], in_=pm_in[:])
            nc.vector.tensor_scalar_add(IP4[:], IP4[:], float(EPS))
            nc.vector.reciprocal(IP4[:], IP4[:])
            nc.vector.tensor_scalar_mul(IP4[:], IP4[:], 0.25)
            Dv = D[:].rearrange("p (b s) -> p b s", s=SLOT)
            IPv = IP4[:].rearrange("p (b s) -> p b s", s=SLOT)

            def eng(name):
                return nc.gpsimd if name == "pool" else nc.vector

            WSB = None
            if V3_PE_WARM:
                WSB = state.tile([1, 1], mybir.dt.float32, tag="wsb")

            for it in range(1, n_iter + 1):
                w = max(WIN - 2 * it, V3_FLOOR)
                a = (WIN - w) // 2 + 1
                if V3_PE_WARM:
                    for _ in range(V3_PE_WARM):
                        PW = ps.tile([WIN, 64], mybir.dt.float32, tag="warm")
                        nc.tensor.matmul(PW[:], TRI[:], TRI[:, 0:64],
                                         start=True, stop=True)
                        if it == n_iter:
                            nc.vector.tensor_copy(WSB[:], PW[0:1, 0:1])
                merged = w <= V3_MERGE_W and BPC * w <= 512
                groups = [(0, BPC, "dve", "dve", "dve")] if merged else [
                    (g * GB, (g + 1) * GB,
                     V3_H4_ENG[g], V3_EFF_ENG[g], V3_MIN_ENG[g])
                    for g in range(G)]
                for gi, (b0, b1, h4e, effe, mine) in enumerate(groups):
                    nb = b1 - b0
                    H4 = tmp.tile([WIN, nb * w], mybir.dt.float32,
                                  tag=f"h{gi}")
                    N = tmp.tile([WIN, nb * w], mybir.dt.float32,
                                 tag=f"n{gi}")
                    H4v = H4[:].rearrange("p (b s) -> p b s", s=w)
                    Nv = N[:].rearrange("p (b s) -> p b s", s=w)
                    PS0 = ps.tile([WIN, nb * w], mybir.dt.float32,
                                  tag=f"v{gi}")

                    eng(h4e).scalar_tensor_tensor(
                        H4v, Dv[:, b0:b1, a - 1:a - 1 + w], 4.0,
                        Dv[:, b0:b1, a + 1:a + 1 + w],
                        op0=mybir.AluOpType.add, op1=mybir.AluOpType.add)
                    nc.tensor.matmul(PS0[:], TRI[:],
                                     Dv[:, b0:b1, a:a + w],
                                     start=True, stop=True)
                    nc.vector.tensor_add(N[:], PS0[:], H4[:])
                    eng(effe).tensor_mul(Nv, Nv, IPv[:, b0:b1, a:a + w])
                    eng(mine).tensor_tensor(
                        Dv[:, b0:b1, a:a + w], Dv[:, b0:b1, a:a + w], Nv,
                        op=mybir.AluOpType.min)

            Dslots = D[:].rearrange("p (b s) -> p b s", s=SLOT)
            nc.sync.dma_start(out=pl_out[:],
                              in_=Dslots[R:R + 1, :, R:R + 1])
            if V3_PE_WARM:
                nc.sync.dma_start(out=warm_out[:], in_=WSB[:])

    nc.compile()
    return nc


def _build_program_v2(n_iter=NUM_ITER):
    """Column-shrinking variant: iteration k only needs cells within
    L-inf distance 50-k of the window center, so the active column band
    shrinks by 2 per iteration (floored at W_FLOOR). Ops use packed
    [101, 8*w] layouts; the vertical sum runs on PE (fp32 tridiag matmul),
    eff on GpSimd at large widths to unload DVE."""
    import concourse.bacc as bacc
    import concourse.tile as tile
    from concourse import mybir

    nc = bacc.Bacc("TRN2", target_bir_lowering=False, debug=False,
                   num_devices=NCORES)
    pm_in = nc.declare_dram_parameter("pmwin", [WIN, WCOLS], mybir.dt.float32,
                                      isOutput=False)
    d0_in = nc.declare_dram_parameter("d0win", [WIN, WCOLS], mybir.dt.float32,
                                      isOutput=False)
    pl_out = nc.declare_dram_parameter("plens", [1, BPC], mybir.dt.float32,
                                       isOutput=True)

    tri_np = np.zeros((WIN, WIN), dtype=np.float32)
    for i in range(WIN):
        if i > 0:
            tri_np[i - 1, i] = 1.0
        if i < WIN - 1:
            tri_np[i + 1, i] = 1.0
    tri_dram = nc.inline_tensor(tri_np, "tri")

    with tile.TileContext(nc) as tc:
        with (
            tc.tile_pool(name="state", bufs=1) as state,
            tc.tile_pool(name="tmp", bufs=2) as tmp,
            tc.tile_pool(name="ps", bufs=2, space="PSUM") as ps,
        ):
            D = state.tile([WIN, WCOLS], mybir.dt.float32)
            IP4 = state.tile([WIN, WCOLS], mybir.dt.float32)
            TRI = state.tile([WIN, WIN], mybir.dt.float32)

            nc.sync.dma_start(out=D[:], in_=d0_in[:])
            nc.sync.dma_start(out=IP4[:], in_=pm_in[:])
            nc.sync.dma_start(out=TRI[:], in_=tri_dram[:])

            nc.vector.tensor_scalar_add(IP4[:], IP4[:], float(EPS))
            nc.vector.reciprocal(IP4[:], IP4[:])
            nc.vector.tensor_scalar_mul(IP4[:], IP4[:], 0.25)

            Dv = D[:].rearrange("p (b s) -> p b s", s=SLOT)
            IPv = IP4[:].rearrange("p (b s) -> p b s", s=SLOT)

            for it in range(1, n_iter + 1):
                w = max(WIN - 2 * it, W_FLOOR)
                a = (WIN - w) // 2 + 1  # active start col within slot
                H4 = tmp.tile([WIN, BPC * w], mybir.dt.float32, tag="h")
                N = tmp.tile([WIN, BPC * w], mybir.dt.float32, tag="n")
                H4v = H4[:].rearrange("p (b s) -> p b s", s=w)
                Nv = N[:].rearrange("p (b s) -> p b s", s=w)

                # H4 = (left + 4) + right
                nc.vector.scalar_tensor_tensor(
                    H4v, Dv[:, :, a - 1:a - 1 + w], 4.0,
                    Dv[:, :, a + 1:a + 1 + w],
                    op0=mybir.AluOpType.add, op1=mybir.AluOpType.add)

                # V = tridiag @ D on PE; split when 8w exceeds a PSUM bank
                if BPC * w > 512:
                    hb = BPC // 2
                    PS0 = ps.tile([WIN, hb * w], mybir.dt.float32, tag="v0")
                    PS1 = ps.tile([WIN, hb * w], mybir.dt.float32, tag="v1")
                    nc.tensor.matmul(PS0[:], TRI[:],
                                     Dv[:, 0:hb, a:a + w],
                                     start=True, stop=True)
                    nc.tensor.matmul(PS1[:], TRI[:],
                                     Dv[:, hb:BPC, a:a + w],
                                     start=True, stop=True)
                    nc.vector.tensor_add(N[:, 0:hb * w], PS0[:],
                                         H4[:, 0:hb * w])
                    nc.vector.tensor_add(N[:, hb * w:], PS1[:],
                                         H4[:, hb * w:])
                else:
                    PS0 = ps.tile([WIN, BPC * w], mybir.dt.float32, tag="v0")
                    nc.tensor.matmul(PS0[:], TRI[:], Dv[:, :, a:a + w],
                                     start=True, stop=True)
                    nc.vector.tensor_add(N[:], PS0[:], H4[:])

                # eff = N * ip4 ; D = min(D, eff)
                eng = nc.gpsimd if w >= POOL_EFF_MIN_W else nc.vector
                eng.tensor_mul(Nv, Nv, IPv[:, :, a:a + w])
                nc.vector.tensor_tensor(Dv[:, :, a:a + w],
                                        Dv[:, :, a:a + w], Nv,
                                        op=mybir.AluOpType.min)

            Dslots = D[:].rearrange("p (b s) -> p b s", s=SLOT)
            nc.sync.dma_start(out=pl_out[:],
                              in_=Dslots[R:R + 1, :, R:R + 1])

    nc.compile()
    return nc


def _build_program(n_iter=NUM_ITER):
    import concourse.bacc as bacc
    import concourse.tile as tile
    from concourse import mybir

    nc = bacc.Bacc("TRN2", target_bir_lowering=False, debug=False,
                   num_devices=NCORES)
    pm_in = nc.declare_dram_parameter("pmwin", [WIN, WCOLS], mybir.dt.float32,
                                      isOutput=False)
    d0_in = nc.declare_dram_parameter("d0win", [WIN, WCOLS], mybir.dt.float32,
                                      isOutput=False)
    pl_out = nc.declare_dram_parameter("plens", [1, BPC], mybir.dt.float32,
                                       isOutput=True)

    tri_np = np.zeros((WIN, WIN), dtype=np.float32)
    for i in range(WIN):
        if i > 0:
            tri_np[i - 1, i] = 1.0
        if i < WIN - 1:
            tri_np[i + 1, i] = 1.0
    tri_dram = nc.inline_tensor(tri_np, "tri")

    with tile.TileContext(nc) as tc:
        with (
            tc.tile_pool(name="state", bufs=1) as state,
            tc.tile_pool(name="tmp", bufs=2) as tmp,
            tc.tile_pool(name="ps", bufs=2, space="PSUM") as ps,
        ):
            D = state.tile([WIN, WCOLS], mybir.dt.float32)
            IP4 = state.tile([WIN, WCOLS], mybir.dt.float32)
            TRI = state.tile([WIN, WIN], mybir.dt.float32)
            HT = state.tile([WIN, WCOLS], mybir.dt.float32)

            nc.sync.dma_start(out=D[:], in_=d0_in[:])
            nc.sync.dma_start(out=IP4[:], in_=pm_in[:])
            nc.sync.dma_start(out=TRI[:], in_=tri_dram[:])

            # ip4 = 0.25 / (pm + eps); guard cols get 0.25/(1+eps) (harmless:
            # D=0 there is self-maintaining since eff >= 0).
            nc.vector.tensor_scalar_add(IP4[:], IP4[:], float(EPS))
            nc.vector.reciprocal(IP4[:], IP4[:])
            nc.vector.tensor_scalar_mul(IP4[:], IP4[:], 0.25)
            nc.vector.memzero(HT[:])

            for _ in range(n_iter):
                # vertical neighbor sum on PE (bit-exact fp32), 2 PSUM banks
                V0 = ps.tile([WIN, 512], mybir.dt.float32, tag="v0")
                V1 = ps.tile([WIN, WCOLS - 512], mybir.dt.float32, tag="v1")
                nc.tensor.matmul(V0[:], TRI[:], D[:, 0:512],
                                 start=True, stop=True)
                nc.tensor.matmul(V1[:], TRI[:], D[:, 512:WCOLS],
                                 start=True, stop=True)
                # horizontal neighbor sum (guard cols are 0)
                nc.vector.tensor_add(HT[:, 1:WCOLS - 1],
                                     D[:, 0:WCOLS - 2], D[:, 2:WCOLS])
                # N = V + H
                N = tmp.tile([WIN, WCOLS], mybir.dt.float32, tag="n")
                nc.vector.tensor_add(N[:, 0:512], V0[:], HT[:, 0:512])
                nc.vector.tensor_add(N[:, 512:WCOLS], V1[:],
                                     HT[:, 512:WCOLS])
                # eff = (N + 4) * ip4 ; D = min(D, eff)
                nc.vector.scalar_tensor_tensor(
                    N[:], N[:], 4.0, IP4[:],
                    op0=mybir.AluOpType.add, op1=mybir.AluOpType.mult)
                nc.vector.tensor_tensor(D[:], D[:], N[:],
                                        op=mybir.AluOpType.min)

            # extract path lengths: D[50, 50 + 104*b] for b in 0..7
            Dslots = D[:].rearrange("p (b s) -> p b s", s=SLOT)
            nc.sync.dma_start(out=pl_out[:],
                              in_=Dslots[R:R + 1, :, R:R + 1])

    nc.compile()
    return nc


def _prepare_core_inputs(pm, start, goal):
    """pm: (BPC,512,512) f32; start/goal: (BPC,2) int64 (already clipped).
    Returns pmwin, d0win tiles of shape (WIN, WCOLS)."""
    pmwin = np.ones((WIN, WCOLS), dtype=np.float32)
    d0win = np.zeros((WIN, WCOLS), dtype=np.float32)
    big = np.float32(H + W)
    for b in range(BPC):
        sr, sc = int(start[b, 0]), int(start[b, 1])
        r0, c0 = sr - R, sc - R
        rlo, rhi = max(0, r0), min(H, r0 + WIN)
        clo, chi = max(0, c0), min(W, c0 + WIN)
        cb = SLOT * b
        pmwin[rlo - r0:rhi - r0, cb + clo - c0:cb + chi - c0] = \
            pm[b, rlo:rhi, clo:chi]
        d0win[rlo - r0:rhi - r0, cb + clo - c0:cb + chi - c0] = big
        glr, glc = int(goal[b, 0]) - r0, int(goal[b, 1]) - c0
        if rlo - r0 <= glr < rhi - r0 and clo - c0 <= glc < chi - c0:
            d0win[glr, cb + glc] = 0.0
    return pmwin, d0win


def kernel(probability_map, start_coords, goal_coords, _trace=False,
           _n_iter=NUM_ITER):
    from concourse.bass_utils import run_bass_kernel_spmd

    pm = np.asarray(probability_map, dtype=np.float32)
    sc_all = np.asarray(start_coords)
    gc_all = np.asarray(goal_coords)
    B = pm.shape[0]
    assert pm.shape == (B_FULL, 1, H, W) and B == B_FULL

    sr = np.clip(sc_all[:, 0], 0, H - 1).astype(np.int64)
    sc = np.clip(sc_all[:, 1], 0, W - 1).astype(np.int64)
    gr = np.clip(gc_all[:, 0], 0, H - 1).astype(np.int64)
    gc = np.clip(gc_all[:, 1], 0, W - 1).astype(np.int64)
    start = np.stack([sr, sc], axis=1)
    goal = np.stack([gr, gc], axis=1)

    if _n_iter not in _COMPILED:
        _COMPILED[_n_iter] = _build_program_v2(_n_iter)
    nc = _COMPILED[_n_iter]

    in_maps = []
    for c in range(NCORES):
        lo = c * BPC
        pmwin, d0win = _prepare_core_inputs(
            pm[lo:lo + BPC, 0], start[lo:lo + BPC], goal[lo:lo + BPC])
        in_maps.append({"pmwin": pmwin, "d0win": d0win})

    res = run_bass_kernel_spmd(nc, in_maps, list(range(NCORES)))
    path_lengths = np.concatenate(
        [np.asarray(r["plens"]).reshape(BPC) for r in res.results])

    diff = (gc_all - sc_all).astype(np.float32)
    euclid = np.sqrt((diff * diff).sum(axis=1, dtype=np.float32))
    euclid = np.maximum(euclid, np.float32(1.0))
    tortuosity = (path_lengths / euclid).astype(np.float32)
    is_valid = path_lengths < np.float32(H + W)
    return tortuosity, is_valid
